# revision 4
# baseline (speedup 1.0000x reference)
"""MPNN (message-passing GNN) Trainium2 kernel, 8-core SPMD — v5.1.

The axon PJRT execute path charges a large per-STATIC-instruction dispatch
cost, so v5 expresses every hot loop as a tc.For_i hardware loop:

  - layer-0 node encoder folded into the layer-0 table/update weights
    (h0 = x@W+b is linear, so tables/updates read x directly) — no preamble.
  - table build: For_i over 98 node ranges.
  - edge pass: nested For_i (98 dst ranges x NB blocks of J=6 tiles of
    128 edges); per-block single DMAs for src/dst (blob [128,2J] i32) and
    edge features ([16, J*128] f8); per-tile indirect gathers into one
    wide SBUF tile; per-block PSUM pattern = zero-matmul wide open, J
    eaC slice accumulations (start before gathers land), ONE wide
    gather-add close, ONE wide relu. PSUM accumulation groups are
    bank-level: never interleave two open start/stop groups in one bank
    (silent corruption). Segment sums via dst one-hot (DVE is_equal)
    matmuls into a per-range accumulator held open across blocks.
  - node update: For_i over 25 tiles of 500 nodes; the second
    message-MLP layer is folded through the segment sum into the update
    weights, with node degree as a 65th feature row.
  - table + edge_attr + gathers in fp8 e4m3 (final rel err ~2e-4 vs the
    2e-2 tolerance) halving AllGather + upload bytes; x uploaded f16;
    per-layer weights packed into single DMAs; sigma-permuted table rows
    so the AllGather input is staged by one contiguous DMA.
"""
import numpy as np
import ml_dtypes

NCORES = 8
N, E, G = 100000, 1600000, 1000
IN_C, EDGE_C, DESC, H, L = 32, 16, 200, 64, 3
NC_N = N // NCORES             # 12500 nodes per core
NRANGES = (NC_N + 127) // 128  # 98
NC_PAD = NRANGES * 128         # 12544
NTAB = NCORES * NC_PAD         # 100352 table rows
NODE_TILE = 500                # 25 x 500 = 12500
N_NODE_TILES = NC_N // NODE_TILE
F8 = ml_dtypes.float8_e4m3


def _host_prep(x, edge_index, edge_attr,
               node_W, node_b, edge_W, edge_b,
               emW1, emb1, emW2, emb2, umW1, umb1, umW2, umb2):
    f32 = np.float32
    f16 = np.float16
    src_all = np.asarray(edge_index[0]).astype(np.int64)
    dst_all = np.asarray(edge_index[1]).astype(np.int64)
    x = np.asarray(x, f32)
    edge_attr = np.asarray(edge_attr, f32)

    emW1 = np.asarray(emW1, f32); emb1 = np.asarray(emb1, f32)
    emW2 = np.asarray(emW2, f32); emb2 = np.asarray(emb2, f32)
    umW1 = np.asarray(umW1, f32); umb1 = np.asarray(umb1, f32)
    umW2 = np.asarray(umW2, f32); umb2 = np.asarray(umb2, f32)
    node_W = np.asarray(node_W, f32); node_b = np.asarray(node_b, f32)
    edge_W = np.asarray(edge_W, f32); edge_b = np.asarray(edge_b, f32)

    # d_l = edge_b @ emW1[l][H:] + emb1[l]  (edge-bias folded into table)
    d = np.stack([edge_b @ emW1[l][H:] + emb1[l] for l in range(L)])
    # layer 0 reads x directly (33 rows: 32 + ones row)
    tabW0 = np.vstack([node_W @ emW1[0][:H],
                       (node_b @ emW1[0][:H] + d[0])[None, :]])     # [33,64]
    updW0 = np.vstack([node_W @ umW1[0][:H],
                       (node_b @ umW1[0][:H])[None, :]])            # [33,64]
    wpack0 = np.concatenate([tabW0, updW0], axis=1).astype(f16)     # [33,128]

    # packed per-layer weights [L, 65, 258] f32:
    # cols 0:64 tabW(+d row), 64:128 W2bx, 128:192 updW, 192:256 umW2,
    # col 256 umb1, col 257 umb2
    wpackL = np.zeros((L, H + 1, 4 * H + 2), f32)
    for l in range(L):
        if l > 0:
            wpackL[l, :, 0:H] = np.vstack([emW1[l][:H], d[l][None, :]])
            wpackL[l, :H, 2 * H:3 * H] = umW1[l][:H]
        wpackL[l, :, H:2 * H] = np.vstack(
            [emW2[l] @ umW1[l][H:], (emb2[l] @ umW1[l][H:])[None, :]])
        wpackL[l, :H, 3 * H:4 * H] = umW2[l]
        wpackL[l, :H, 4 * H] = umb1[l]
        wpackL[l, :H, 4 * H + 1] = umb2[l]
    Cmat = np.stack([edge_W @ emW1[l][H:] for l in range(L)]).astype(F8)

    deg = np.bincount(dst_all, minlength=N).astype(f32)

    core_of = dst_all // NC_N
    per_core = []
    counts = np.zeros((NCORES, NRANGES), np.int64)
    for c in range(NCORES):
        sel = np.nonzero(core_of == c)[0]
        d_loc = dst_all[sel] - c * NC_N
        order = np.argsort(d_loc, kind="stable")
        sel = sel[order]
        d_loc = d_loc[order]
        per_core.append((sel, d_loc))
        counts[c] = np.bincount(d_loc // 128, minlength=NRANGES)

    J = 6
    Tc = int(np.ceil(counts.max() / 128))        # tiles per range, uniform
    Tc = ((Tc + J - 1) // J) * J                 # round up to J-multiple
    NB = Tc // J
    n_slots = NRANGES * Tc * 128

    in_maps = []
    for c in range(NCORES):
        sel, d_loc = per_core[c]
        rng_idx = d_loc // 128
        range_start = np.concatenate([[0], np.cumsum(counts[c])])[:-1]
        k = np.arange(len(sel)) - range_start[rng_idx]
        slots = ((rng_idx * Tc) * 128 + k).astype(np.int64)

        src_arr = np.zeros(n_slots, np.int32)
        dst_arr = np.full(n_slots, -1, np.int32)
        sg = src_all[sel]
        si = sg % NC_N
        src_arr[slots] = ((sg // NC_N) * NC_PAD + (si % 128) * NRANGES
                          + si // 128).astype(np.int32)
        dst_arr[slots] = (d_loc - rng_idx * 128).astype(np.int32)
        ea_arr = np.zeros((n_slots, EDGE_C), f32)
        ea_arr[slots] = edge_attr[sel]

        # block layout: NB blocks of J tiles per range
        # blob [NRANGES*NB, 128, 2J]: cols 0..J-1 = src_j, J..2J-1 = dst_j
        sr = src_arr.reshape(NRANGES * NB, J, 128)
        dr = dst_arr.reshape(NRANGES * NB, J, 128)
        blob = np.concatenate([sr.transpose(0, 2, 1),
                               dr.transpose(0, 2, 1)], axis=2).copy()
        # eaTr [NRANGES*NB, 16, J*128]
        eaTr = ea_arr.reshape(NRANGES * NB, J, 128, EDGE_C) \
                     .transpose(0, 3, 1, 2) \
                     .reshape(NRANGES * NB, EDGE_C, J * 128).astype(F8).copy()

        xT1 = np.zeros((IN_C + 1, NC_PAD), f16)
        xT1[:IN_C, :NC_N] = x[c * NC_N:(c + 1) * NC_N].T.astype(f16)
        xT1[IN_C, :] = 1.0

        degp = np.zeros((1, NC_PAD), f32)
        degp[0, :NC_N] = deg[c * NC_N:(c + 1) * NC_N]

        in_maps.append({
            "xT1": xT1,
            "degr": degp,
            "blob": blob, "eaTr": eaTr,
            "w_pack0": wpack0, "w_packL": wpackL, "w_C": Cmat,
        })
    meta = dict(Tc=Tc, J=J, NB=NB)
    return in_maps, meta


def _build(meta, reps=1, ag=True, unroll=1, gather=True, tiledma=True):
    import concourse.bass as bass
    from concourse.bass import ds
    import concourse.mybir as mybir
    import concourse.tile as tile
    from concourse import bacc

    Tc, J, NB = meta["Tc"], meta["J"], meta["NB"]
    f32 = mybir.dt.float32
    f16 = mybir.dt.float16
    f8 = mybir.dt.float8e4
    i32 = mybir.dt.int32

    nc = bacc.Bacc("TRN2", target_bir_lowering=False, debug=False,
                   num_devices=NCORES)
    xT1 = nc.dram_tensor("xT1", [IN_C + 1, NC_PAD], f16, kind="ExternalInput")
    degr = nc.dram_tensor("degr", [1, NC_PAD], f32, kind="ExternalInput")
    blobD = nc.dram_tensor("blob", [NRANGES * NB, 128, 2 * J], i32,
                           kind="ExternalInput")
    eaTD = nc.dram_tensor("eaTr", [NRANGES * NB, EDGE_C, J * 128], f8,
                          kind="ExternalInput")
    w_pack0 = nc.dram_tensor("w_pack0", [IN_C + 1, 2 * H], f16,
                             kind="ExternalInput")
    w_packL = nc.dram_tensor("w_packL", [L, H + 1, 4 * H + 2], f32,
                             kind="ExternalInput")
    w_C = nc.dram_tensor("w_C", [L, EDGE_C, H], f8, kind="ExternalInput")
    hT_out = nc.dram_tensor("hT_out", [H, NC_N], f32, kind="ExternalOutput")

    Relu = mybir.ActivationFunctionType.Relu

    with tile.TileContext(nc) as tc:
        with (
            tc.tile_pool(name="const", bufs=1) as constp,
            tc.tile_pool(name="wts", bufs=2) as wp,
            tc.tile_pool(name="hbig", bufs=1) as hp,
            tc.tile_pool(name="sp", bufs=4) as sp,
            tc.tile_pool(name="rng", bufs=3) as rp,
            tc.tile_pool(name="nps", bufs=1, space="PSUM") as npsum,
            tc.tile_pool(name="eps", bufs=2, space="PSUM") as epsum,
            tc.tile_pool(name="hps", bufs=1, space="PSUM") as hpsum,
            tc.tile_pool(name="gps", bufs=2, space="PSUM") as gpsum,
            tc.tile_pool(name="dram", bufs=1, space="DRAM") as dramp,
        ):
            iota_row = constp.tile([128, 128], f32)
            nc.gpsimd.iota(iota_row[:], pattern=[[1, 128]], channel_multiplier=0,
                           allow_small_or_imprecise_dtypes=True)
            pcol = constp.tile([128, 1], f32)
            nc.gpsimd.iota(pcol[:], pattern=[[0, 1]], channel_multiplier=1,
                           allow_small_or_imprecise_dtypes=True)
            ident8 = constp.tile([128, 128], f8)
            nc.vector.tensor_scalar(out=ident8[:], in0=iota_row[:],
                                    scalar1=pcol[:, 0:1], scalar2=None,
                                    op0=mybir.AluOpType.is_equal)
            zrow = constp.tile([1, 768], f16)
            nc.vector.memset(zrow[:], 0.0)

            hA = hp.tile([H + 1, NC_PAD], f32, tag="hA")
            hB = hp.tile([H + 1, NC_PAD], f32, tag="hB")
            nc.vector.memset(hA[:], 0.0)
            nc.vector.memset(hB[:], 0.0)
            nc.vector.memset(hA[H:H + 1, :], 1.0)
            nc.vector.memset(hB[H:H + 1, :], 1.0)
            aggTx = hp.tile([H + 1, NC_PAD], f32, tag="agg")
            nc.sync.dma_start(aggTx[H:H + 1, :], degr[:])

            wp0 = constp.tile([IN_C + 1, 2 * H], f16)
            nc.sync.dma_start(wp0[:], w_pack0[:])

            stageD = dramp.tile([NC_PAD, H], f8, tag="stage")
            stage_sb = hp.tile([128, NRANGES * H], f8, tag="stage_sb")

            h_cur, h_nxt = hA, hB
            for rep, l in [(rp_, l_) for rp_ in range(reps)
                           for l_ in range(L)]:
                table = dramp.tile([NTAB, H], f8, addr_space="Shared",
                                   tag=f"table{rep}_{l}")
                wl = wp.tile([H + 1, 4 * H + 2], f32, tag="wl")
                nc.sync.dma_start(wl[:], w_packL[l])
                C_t = wp.tile([EDGE_C, H], f8, tag="C")
                nc.sync.dma_start(C_t[:], w_C[l])

                hrows = IN_C + 1 if l == 0 else H + 1

                # ---- table build: For_i over ranges ----
                with tc.For_i(0, NRANGES, 1) as r:
                    hwp = hpsum.tile([128, H], f32, tag="hw")
                    if l == 0:
                        hsl = sp.tile([IN_C + 1, 128], f16, tag="hsl0")
                        nc.sync.dma_start(hsl[:], xT1[:, ds(r * 128, 128)])
                        nc.tensor.matmul(hwp[:], lhsT=hsl[:],
                                         rhs=wp0[:, 0:H],
                                         start=True, stop=True)
                    else:
                        hsl = sp.tile([H + 1, 128], f32, tag="hsl1")
                        nc.vector.tensor_copy(hsl[:],
                                              h_cur[:, ds(r * 128, 128)])
                        nc.tensor.matmul(hwp[:], lhsT=hsl[:], rhs=wl[:, 0:H],
                                         start=True, stop=True)
                    nc.vector.tensor_copy(stage_sb[:, ds(r * H, H)], hwp[:])

                nc.sync.dma_start(stageD[:], stage_sb[:])
                if ag:
                    nc.gpsimd.collective_compute(
                        "AllGather", mybir.AluOpType.bypass,
                        replica_groups=[list(range(NCORES))],
                        ins=[stageD.opt()], outs=[table.opt()],
                    )
                else:
                    tb_in = dramp.tile([1, 8], f32, tag=f"tbin{rep}_{l}")
                    tb_out = dramp.tile([1, 8], f32, addr_space="Shared",
                                        tag=f"tbout{rep}_{l}")
                    tk = sp.tile([1, 8], f32, tag="tk")
                    nc.vector.memset(tk[:], 1.0)
                    nc.sync.dma_start(tb_in[:], tk[:])
                    nc.gpsimd.collective_compute(
                        "AllReduce", mybir.AluOpType.add,
                        replica_groups=[list(range(NCORES))],
                        ins=[tb_in.opt()], outs=[tb_out.opt()],
                    )

                # ---- edge pass: nested For_i over ranges x tiles ----
                with tc.For_i(0, NRANGES, 1) as r:
                    aps = epsum.tile([H, 128], f32, tag="aps")
                    nc.tensor.matmul(aps[:], lhsT=zrow[0:1, 0:H],
                                     rhs=zrow[0:1, 0:128], start=True,
                                     stop=False)
                    with tc.For_i(0, NB, 1) as t:
                        blobB = rp.tile([128, 2 * J], i32, tag="blobB")
                        nc.sync.dma_start(blobB[:], blobD[ds(r * NB + t, 1)])
                        dstf = rp.tile([128, J], f32, tag="dstf")
                        nc.vector.tensor_copy(dstf[:], blobB[:, J:2 * J])
                        eaB = rp.tile([EDGE_C, J * 128], f8, tag="eaB")
                        nc.sync.dma_start(eaB[:], eaTD[ds(r * NB + t, 1)])
                        gtB = sp.tile([128, J * H], f8, tag="gtB")
                        for j in range(J):
                            nc.gpsimd.indirect_dma_start(
                                out=gtB[:, j * H:(j + 1) * H], out_offset=None,
                                in_=table[:],
                                in_offset=bass.IndirectOffsetOnAxis(
                                    ap=blobB[:, j:j + 1], axis=0))
                        pse = gpsum.tile([128, J * H], f32, tag="pse")
                        # ONE open group per bank: zero-matmul wide open, eaC
                        # slice accumulates (no gather dep), ONE wide
                        # gather-add close.
                        nc.tensor.matmul(pse[:], lhsT=zrow[0:1, 0:128],
                                         rhs=zrow[0:1, 0:J * H],
                                         start=True, stop=False)
                        for j in range(J):
                            nc.tensor.matmul(pse[:, j * H:(j + 1) * H],
                                             lhsT=eaB[:, j * 128:(j + 1) * 128],
                                             rhs=C_t[:],
                                             start=False, stop=False)
                        nc.tensor.matmul(pse[:], lhsT=ident8[:], rhs=gtB[:],
                                         start=False, stop=True)
                        rlB = sp.tile([128, J * H], f16, tag="rlB")
                        nc.scalar.activation(rlB[:], pse[:], Relu)
                        for j in range(J):
                            oh = sp.tile([128, 128], f16, tag=f"oh{j}")
                            nc.vector.tensor_scalar(
                                out=oh[:], in0=iota_row[:],
                                scalar1=dstf[:, j:j + 1], scalar2=None,
                                op0=mybir.AluOpType.is_equal)
                            nc.tensor.matmul(aps[:],
                                             lhsT=rlB[:, j * H:(j + 1) * H],
                                             rhs=oh[:],
                                             start=False, stop=False)
                    nc.tensor.matmul(aps[:], lhsT=zrow[0:1, 0:H],
                                     rhs=zrow[0:1, 0:128], start=False,
                                     stop=True)
                    nc.vector.tensor_copy(aggTx[:H, ds(r * 128, 128)], aps[:])

                # ---- node update: For_i over 25 tiles of 500 ----
                with tc.For_i(0, N_NODE_TILES, 1) as u:
                    ps = npsum.tile([H, NODE_TILE], f32, tag="ps1")
                    if l == 0:
                        xu = sp.tile([IN_C + 1, NODE_TILE], f16, tag="xu")
                        nc.sync.dma_start(xu[:],
                                          xT1[:, ds(u * NODE_TILE, NODE_TILE)])
                        nc.tensor.matmul(ps[:], lhsT=wp0[:, H:2 * H],
                                         rhs=xu[:], start=True, stop=False)
                    else:
                        nc.tensor.matmul(
                            ps[:], lhsT=wl[:H, 2 * H:3 * H],
                            rhs=h_cur[:H, ds(u * NODE_TILE, NODE_TILE)],
                            start=True, stop=False)
                    nc.tensor.matmul(ps[:], lhsT=wl[:, H:2 * H],
                                     rhs=aggTx[:, ds(u * NODE_TILE, NODE_TILE)],
                                     start=False, stop=True)
                    rl1 = sp.tile([H, NODE_TILE], f32, tag="nrl")
                    nc.scalar.activation(rl1[:], ps[:], Relu,
                                         bias=wl[:H, 4 * H:4 * H + 1])
                    ps2 = npsum.tile([H, NODE_TILE], f32, tag="ps2")
                    nc.tensor.matmul(ps2[:], lhsT=wl[:H, 3 * H:4 * H],
                                     rhs=rl1[:], start=True, stop=True)
                    nc.vector.tensor_scalar_add(
                        h_nxt[:H, ds(u * NODE_TILE, NODE_TILE)], ps2[:],
                        wl[:H, 4 * H + 1:4 * H + 2])
                h_cur, h_nxt = h_nxt, h_cur

            nc.sync.dma_start(hT_out[:], h_cur[:H, :NC_N])
    nc.compile()
    return nc


def kernel(**inputs):
    from concourse.bass_utils import run_bass_kernel_spmd

    ro_W1 = np.asarray(inputs["ro_W1"], np.float32)
    ro_b1 = np.asarray(inputs["ro_b1"], np.float32)
    ro_W2 = np.asarray(inputs["ro_W2"], np.float32)
    ro_b2 = np.asarray(inputs["ro_b2"], np.float32)
    batch = np.asarray(inputs["batch"]).astype(np.int64)
    descriptors = np.asarray(inputs["descriptors"], np.float32)

    in_maps, meta = _host_prep(
        inputs["x"], inputs["edge_index"], inputs["edge_attr"],
        inputs["node_W"], inputs["node_b"], inputs["edge_W"], inputs["edge_b"],
        inputs["emW1"], inputs["emb1"], inputs["emW2"], inputs["emb2"],
        inputs["umW1"], inputs["umb1"], inputs["umW2"], inputs["umb2"])

    nc = _build(meta)

    res = run_bass_kernel_spmd(nc, in_maps, core_ids=list(range(NCORES)))

    h = np.concatenate([res.results[c]["hT_out"].T for c in range(NCORES)],
                       axis=0)

    # host readout: mean-pool per graph + MLP + sigmoid (0.05% of FLOPs)
    sums = np.zeros((G, H), np.float32)
    np.add.at(sums, batch, h)
    cnt = np.bincount(batch, minlength=G).astype(np.float32)
    pooled = sums / np.maximum(cnt, 1.0)[:, None]
    r = np.concatenate([pooled, descriptors], axis=1)
    z = np.maximum(r @ ro_W1 + ro_b1, 0.0) @ ro_W2 + ro_b2
    out = 1.0 / (1.0 + np.exp(-z))
    return out.reshape(-1).astype(np.float32)


# revision 5
# speedup vs baseline: 1.8265x; 1.8265x over previous
"""MPNN (message-passing GNN) Trainium2 kernel, 8-core SPMD — v5.1.

The axon PJRT execute path charges a large per-STATIC-instruction dispatch
cost, so v5 expresses every hot loop as a tc.For_i hardware loop:

  - layer-0 node encoder folded into the layer-0 table/update weights
    (h0 = x@W+b is linear, so tables/updates read x directly) — no preamble.
  - table build: For_i over 98 node ranges.
  - edge pass: nested For_i (49 range-PAIRS x NB blocks of J=6 tiles of
    128 edges) — two independent per-range dependency chains per
    iteration for ILP; per-block single DMAs for src/dst (blob [128,2J]
    i32) and edge features ([16, J*128] f8); per-tile indirect gathers
    into one wide SBUF tile; per-block PSUM pattern = zero-matmul wide
    open, J eaC slice accumulations (start before gathers land), ONE
    wide gather-add close, ONE wide relu. PSUM accumulation groups are
    bank-level: never interleave two open start/stop groups in one bank
    (silent corruption). Segment sums via dst one-hot (DVE is_equal)
    matmuls into a per-range accumulator held open across blocks.
  - node update: For_i over 25 tiles of 500 nodes; the second
    message-MLP layer is folded through the segment sum into the update
    weights, with node degree as a 65th feature row.
  - table + edge_attr + gathers in fp8 e4m3 (final rel err ~2e-4 vs the
    2e-2 tolerance) halving AllGather + upload bytes; x uploaded f16;
    per-layer weights packed into single DMAs; sigma-permuted table rows
    so the AllGather input is staged by one contiguous DMA.
"""
import numpy as np
import ml_dtypes

NCORES = 8
N, E, G = 100000, 1600000, 1000
IN_C, EDGE_C, DESC, H, L = 32, 16, 200, 64, 3
NC_N = N // NCORES             # 12500 nodes per core
NRANGES = (NC_N + 127) // 128  # 98
NC_PAD = NRANGES * 128         # 12544
NTAB = NCORES * NC_PAD         # 100352 table rows
NODE_TILE = 500                # 25 x 500 = 12500
N_NODE_TILES = NC_N // NODE_TILE
F8 = ml_dtypes.float8_e4m3


def _host_prep(x, edge_index, edge_attr,
               node_W, node_b, edge_W, edge_b,
               emW1, emb1, emW2, emb2, umW1, umb1, umW2, umb2):
    f32 = np.float32
    f16 = np.float16
    src_all = np.asarray(edge_index[0]).astype(np.int64)
    dst_all = np.asarray(edge_index[1]).astype(np.int64)
    x = np.asarray(x, f32)
    edge_attr = np.asarray(edge_attr, f32)

    emW1 = np.asarray(emW1, f32); emb1 = np.asarray(emb1, f32)
    emW2 = np.asarray(emW2, f32); emb2 = np.asarray(emb2, f32)
    umW1 = np.asarray(umW1, f32); umb1 = np.asarray(umb1, f32)
    umW2 = np.asarray(umW2, f32); umb2 = np.asarray(umb2, f32)
    node_W = np.asarray(node_W, f32); node_b = np.asarray(node_b, f32)
    edge_W = np.asarray(edge_W, f32); edge_b = np.asarray(edge_b, f32)

    # d_l = edge_b @ emW1[l][H:] + emb1[l]  (edge-bias folded into table)
    d = np.stack([edge_b @ emW1[l][H:] + emb1[l] for l in range(L)])
    # layer 0 reads x directly (33 rows: 32 + ones row)
    tabW0 = np.vstack([node_W @ emW1[0][:H],
                       (node_b @ emW1[0][:H] + d[0])[None, :]])     # [33,64]
    updW0 = np.vstack([node_W @ umW1[0][:H],
                       (node_b @ umW1[0][:H])[None, :]])            # [33,64]
    wpack0 = np.concatenate([tabW0, updW0], axis=1).astype(f16)     # [33,128]

    # packed per-layer weights [L, 65, 258] f32:
    # cols 0:64 tabW(+d row), 64:128 W2bx, 128:192 updW, 192:256 umW2,
    # col 256 umb1, col 257 umb2
    wpackL = np.zeros((L, H + 1, 4 * H + 2), f32)
    for l in range(L):
        if l > 0:
            wpackL[l, :, 0:H] = np.vstack([emW1[l][:H], d[l][None, :]])
            wpackL[l, :H, 2 * H:3 * H] = umW1[l][:H]
        wpackL[l, :, H:2 * H] = np.vstack(
            [emW2[l] @ umW1[l][H:], (emb2[l] @ umW1[l][H:])[None, :]])
        wpackL[l, :H, 3 * H:4 * H] = umW2[l]
        wpackL[l, :H, 4 * H] = umb1[l]
        wpackL[l, :H, 4 * H + 1] = umb2[l]
    Cmat = np.stack([edge_W @ emW1[l][H:] for l in range(L)]).astype(F8)

    deg = np.bincount(dst_all, minlength=N).astype(f32)

    core_of = dst_all // NC_N
    per_core = []
    counts = np.zeros((NCORES, NRANGES), np.int64)
    for c in range(NCORES):
        sel = np.nonzero(core_of == c)[0]
        d_loc = dst_all[sel] - c * NC_N
        order = np.argsort(d_loc, kind="stable")
        sel = sel[order]
        d_loc = d_loc[order]
        per_core.append((sel, d_loc))
        counts[c] = np.bincount(d_loc // 128, minlength=NRANGES)

    J = 6
    Tc = int(np.ceil(counts.max() / 128))        # tiles per range, uniform
    Tc = ((Tc + J - 1) // J) * J                 # round up to J-multiple
    NB = Tc // J
    n_slots = NRANGES * Tc * 128

    in_maps = []
    for c in range(NCORES):
        sel, d_loc = per_core[c]
        rng_idx = d_loc // 128
        range_start = np.concatenate([[0], np.cumsum(counts[c])])[:-1]
        k = np.arange(len(sel)) - range_start[rng_idx]
        slots = ((rng_idx * Tc) * 128 + k).astype(np.int64)

        src_arr = np.zeros(n_slots, np.int32)
        dst_arr = np.full(n_slots, -1, np.int32)
        sg = src_all[sel]
        si = sg % NC_N
        src_arr[slots] = ((sg // NC_N) * NC_PAD + (si % 128) * NRANGES
                          + si // 128).astype(np.int32)
        dst_arr[slots] = (d_loc - rng_idx * 128).astype(np.int32)
        ea_arr = np.zeros((n_slots, EDGE_C), f32)
        ea_arr[slots] = edge_attr[sel]

        # block layout: NB blocks of J tiles per range
        # blob [NRANGES*NB, 128, 2J]: cols 0..J-1 = src_j, J..2J-1 = dst_j
        sr = src_arr.reshape(NRANGES * NB, J, 128)
        dr = dst_arr.reshape(NRANGES * NB, J, 128)
        blob = np.concatenate([sr.transpose(0, 2, 1),
                               dr.transpose(0, 2, 1)], axis=2).copy()
        # eaTr [NRANGES*NB, 16, J*128]
        eaTr = ea_arr.reshape(NRANGES * NB, J, 128, EDGE_C) \
                     .transpose(0, 3, 1, 2) \
                     .reshape(NRANGES * NB, EDGE_C, J * 128).astype(F8).copy()

        xT1 = np.zeros((IN_C + 1, NC_PAD), f16)
        xT1[:IN_C, :NC_N] = x[c * NC_N:(c + 1) * NC_N].T.astype(f16)
        xT1[IN_C, :] = 1.0

        degp = np.zeros((1, NC_PAD), f32)
        degp[0, :NC_N] = deg[c * NC_N:(c + 1) * NC_N]

        in_maps.append({
            "xT1": xT1,
            "degr": degp,
            "blob": blob, "eaTr": eaTr,
            "w_pack0": wpack0, "w_packL": wpackL, "w_C": Cmat,
        })
    meta = dict(Tc=Tc, J=J, NB=NB)
    return in_maps, meta


def _build(meta, reps=1, ag=True, unroll=1, gather=True, tiledma=True):
    import concourse.bass as bass
    from concourse.bass import ds
    import concourse.mybir as mybir
    import concourse.tile as tile
    from concourse import bacc

    Tc, J, NB = meta["Tc"], meta["J"], meta["NB"]
    f32 = mybir.dt.float32
    f16 = mybir.dt.float16
    f8 = mybir.dt.float8e4
    i32 = mybir.dt.int32

    nc = bacc.Bacc("TRN2", target_bir_lowering=False, debug=False,
                   num_devices=NCORES)
    xT1 = nc.dram_tensor("xT1", [IN_C + 1, NC_PAD], f16, kind="ExternalInput")
    degr = nc.dram_tensor("degr", [1, NC_PAD], f32, kind="ExternalInput")
    blobD = nc.dram_tensor("blob", [NRANGES * NB, 128, 2 * J], i32,
                           kind="ExternalInput")
    eaTD = nc.dram_tensor("eaTr", [NRANGES * NB, EDGE_C, J * 128], f8,
                          kind="ExternalInput")
    w_pack0 = nc.dram_tensor("w_pack0", [IN_C + 1, 2 * H], f16,
                             kind="ExternalInput")
    w_packL = nc.dram_tensor("w_packL", [L, H + 1, 4 * H + 2], f32,
                             kind="ExternalInput")
    w_C = nc.dram_tensor("w_C", [L, EDGE_C, H], f8, kind="ExternalInput")
    hT_out = nc.dram_tensor("hT_out", [H, NC_N], f32, kind="ExternalOutput")

    Relu = mybir.ActivationFunctionType.Relu

    with tile.TileContext(nc) as tc:
        with (
            tc.tile_pool(name="const", bufs=1) as constp,
            tc.tile_pool(name="wts", bufs=2) as wp,
            tc.tile_pool(name="hbig", bufs=1) as hp,
            tc.tile_pool(name="sp", bufs=4) as sp,
            tc.tile_pool(name="rng", bufs=3) as rp,
            tc.tile_pool(name="nps", bufs=1, space="PSUM") as npsum,
            tc.tile_pool(name="eps", bufs=1, space="PSUM") as epsum,
            tc.tile_pool(name="hps", bufs=1, space="PSUM") as hpsum,
            tc.tile_pool(name="gps", bufs=2, space="PSUM") as gpsum,
            tc.tile_pool(name="dram", bufs=1, space="DRAM") as dramp,
        ):
            iota_row = constp.tile([128, 128], f32)
            nc.gpsimd.iota(iota_row[:], pattern=[[1, 128]], channel_multiplier=0,
                           allow_small_or_imprecise_dtypes=True)
            pcol = constp.tile([128, 1], f32)
            nc.gpsimd.iota(pcol[:], pattern=[[0, 1]], channel_multiplier=1,
                           allow_small_or_imprecise_dtypes=True)
            ident8 = constp.tile([128, 128], f8)
            nc.vector.tensor_scalar(out=ident8[:], in0=iota_row[:],
                                    scalar1=pcol[:, 0:1], scalar2=None,
                                    op0=mybir.AluOpType.is_equal)
            zrow = constp.tile([1, 768], f16)
            nc.vector.memset(zrow[:], 0.0)

            hA = hp.tile([H + 1, NC_PAD], f32, tag="hA")
            hB = hp.tile([H + 1, NC_PAD], f32, tag="hB")
            nc.vector.memset(hA[:], 0.0)
            nc.vector.memset(hB[:], 0.0)
            nc.vector.memset(hA[H:H + 1, :], 1.0)
            nc.vector.memset(hB[H:H + 1, :], 1.0)
            aggTx = hp.tile([H + 1, NC_PAD], f32, tag="agg")
            nc.sync.dma_start(aggTx[H:H + 1, :], degr[:])

            wp0 = constp.tile([IN_C + 1, 2 * H], f16)
            nc.sync.dma_start(wp0[:], w_pack0[:])

            stageD = dramp.tile([NC_PAD, H], f8, tag="stage")
            stage_sb = hp.tile([128, NRANGES * H], f8, tag="stage_sb")

            h_cur, h_nxt = hA, hB
            for rep, l in [(rp_, l_) for rp_ in range(reps)
                           for l_ in range(L)]:
                table = dramp.tile([NTAB, H], f8, addr_space="Shared",
                                   tag=f"table{rep}_{l}")
                wl = wp.tile([H + 1, 4 * H + 2], f32, tag="wl")
                nc.sync.dma_start(wl[:], w_packL[l])
                C_t = wp.tile([EDGE_C, H], f8, tag="C")
                nc.sync.dma_start(C_t[:], w_C[l])

                hrows = IN_C + 1 if l == 0 else H + 1

                # ---- table build: For_i over ranges ----
                with tc.For_i(0, NRANGES, 1) as r:
                    hwp = hpsum.tile([128, H], f32, tag="hw")
                    if l == 0:
                        hsl = sp.tile([IN_C + 1, 128], f16, tag="hsl0")
                        nc.sync.dma_start(hsl[:], xT1[:, ds(r * 128, 128)])
                        nc.tensor.matmul(hwp[:], lhsT=hsl[:],
                                         rhs=wp0[:, 0:H],
                                         start=True, stop=True)
                    else:
                        hsl = sp.tile([H + 1, 128], f32, tag="hsl1")
                        nc.vector.tensor_copy(hsl[:],
                                              h_cur[:, ds(r * 128, 128)])
                        nc.tensor.matmul(hwp[:], lhsT=hsl[:], rhs=wl[:, 0:H],
                                         start=True, stop=True)
                    nc.vector.tensor_copy(stage_sb[:, ds(r * H, H)], hwp[:])

                nc.sync.dma_start(stageD[:], stage_sb[:])
                if ag:
                    nc.gpsimd.collective_compute(
                        "AllGather", mybir.AluOpType.bypass,
                        replica_groups=[list(range(NCORES))],
                        ins=[stageD.opt()], outs=[table.opt()],
                    )
                else:
                    tb_in = dramp.tile([1, 8], f32, tag=f"tbin{rep}_{l}")
                    tb_out = dramp.tile([1, 8], f32, addr_space="Shared",
                                        tag=f"tbout{rep}_{l}")
                    tk = sp.tile([1, 8], f32, tag="tk")
                    nc.vector.memset(tk[:], 1.0)
                    nc.sync.dma_start(tb_in[:], tk[:])
                    nc.gpsimd.collective_compute(
                        "AllReduce", mybir.AluOpType.add,
                        replica_groups=[list(range(NCORES))],
                        ins=[tb_in.opt()], outs=[tb_out.opt()],
                    )

                # ---- edge pass: For_i over range PAIRS x blocks ----
                # two independent per-range chains per iteration for ILP
                with tc.For_i(0, NRANGES // 2, 1) as rr:
                    apss = []
                    for h_ in range(2):
                        aps = epsum.tile([H, 128], f32, tag=f"aps{h_}")
                        apss.append(aps)
                        nc.tensor.matmul(aps[:], lhsT=zrow[0:1, 0:H],
                                         rhs=zrow[0:1, 0:128], start=True,
                                         stop=False)
                    with tc.For_i(0, NB, 1) as t:
                        for h_ in range(2):
                            bi = (rr * 2 + h_) * NB + t
                            blobB = rp.tile([128, 2 * J], i32, tag=f"blobB{h_}")
                            nc.sync.dma_start(blobB[:], blobD[ds(bi, 1)])
                            dstf = rp.tile([128, J], f32, tag=f"dstf{h_}")
                            nc.vector.tensor_copy(dstf[:], blobB[:, J:2 * J])
                            eaB = rp.tile([EDGE_C, J * 128], f8, tag=f"eaB{h_}")
                            nc.sync.dma_start(eaB[:], eaTD[ds(bi, 1)])
                            gtB = sp.tile([128, J * H], f8, tag=f"gtB{h_}")
                            for j in range(J):
                                nc.gpsimd.indirect_dma_start(
                                    out=gtB[:, j * H:(j + 1) * H],
                                    out_offset=None,
                                    in_=table[:],
                                    in_offset=bass.IndirectOffsetOnAxis(
                                        ap=blobB[:, j:j + 1], axis=0))
                            pse = gpsum.tile([128, J * H], f32, tag="pse")
                            # ONE open group per bank: zero-matmul wide open,
                            # eaC slice accumulates (no gather dep), ONE wide
                            # gather-add close.
                            nc.tensor.matmul(pse[:], lhsT=zrow[0:1, 0:128],
                                             rhs=zrow[0:1, 0:J * H],
                                             start=True, stop=False)
                            for j in range(J):
                                nc.tensor.matmul(
                                    pse[:, j * H:(j + 1) * H],
                                    lhsT=eaB[:, j * 128:(j + 1) * 128],
                                    rhs=C_t[:], start=False, stop=False)
                            nc.tensor.matmul(pse[:], lhsT=ident8[:],
                                             rhs=gtB[:], start=False,
                                             stop=True)
                            rlB = sp.tile([128, J * H], f16, tag=f"rlB{h_}")
                            nc.scalar.activation(rlB[:], pse[:], Relu)
                            for j in range(J):
                                oh = sp.tile([128, 128], f16,
                                             tag=f"oh{h_}_{j}")
                                nc.vector.tensor_scalar(
                                    out=oh[:], in0=iota_row[:],
                                    scalar1=dstf[:, j:j + 1], scalar2=None,
                                    op0=mybir.AluOpType.is_equal)
                                nc.tensor.matmul(
                                    apss[h_][:],
                                    lhsT=rlB[:, j * H:(j + 1) * H],
                                    rhs=oh[:], start=False, stop=False)
                    for h_ in range(2):
                        nc.tensor.matmul(apss[h_][:], lhsT=zrow[0:1, 0:H],
                                         rhs=zrow[0:1, 0:128], start=False,
                                         stop=True)
                        nc.vector.tensor_copy(
                            aggTx[:H, ds((rr * 2 + h_) * 128, 128)],
                            apss[h_][:])

                # ---- node update: For_i over 25 tiles of 500 ----
                with tc.For_i(0, N_NODE_TILES, 1) as u:
                    ps = npsum.tile([H, NODE_TILE], f32, tag="ps1")
                    if l == 0:
                        xu = sp.tile([IN_C + 1, NODE_TILE], f16, tag="xu")
                        nc.sync.dma_start(xu[:],
                                          xT1[:, ds(u * NODE_TILE, NODE_TILE)])
                        nc.tensor.matmul(ps[:], lhsT=wp0[:, H:2 * H],
                                         rhs=xu[:], start=True, stop=False)
                    else:
                        nc.tensor.matmul(
                            ps[:], lhsT=wl[:H, 2 * H:3 * H],
                            rhs=h_cur[:H, ds(u * NODE_TILE, NODE_TILE)],
                            start=True, stop=False)
                    nc.tensor.matmul(ps[:], lhsT=wl[:, H:2 * H],
                                     rhs=aggTx[:, ds(u * NODE_TILE, NODE_TILE)],
                                     start=False, stop=True)
                    rl1 = sp.tile([H, NODE_TILE], f32, tag="nrl")
                    nc.scalar.activation(rl1[:], ps[:], Relu,
                                         bias=wl[:H, 4 * H:4 * H + 1])
                    ps2 = npsum.tile([H, NODE_TILE], f32, tag="ps2")
                    nc.tensor.matmul(ps2[:], lhsT=wl[:H, 3 * H:4 * H],
                                     rhs=rl1[:], start=True, stop=True)
                    nc.vector.tensor_scalar_add(
                        h_nxt[:H, ds(u * NODE_TILE, NODE_TILE)], ps2[:],
                        wl[:H, 4 * H + 1:4 * H + 2])
                h_cur, h_nxt = h_nxt, h_cur

            nc.sync.dma_start(hT_out[:], h_cur[:H, :NC_N])
    nc.compile()
    return nc


def kernel(**inputs):
    from concourse.bass_utils import run_bass_kernel_spmd

    ro_W1 = np.asarray(inputs["ro_W1"], np.float32)
    ro_b1 = np.asarray(inputs["ro_b1"], np.float32)
    ro_W2 = np.asarray(inputs["ro_W2"], np.float32)
    ro_b2 = np.asarray(inputs["ro_b2"], np.float32)
    batch = np.asarray(inputs["batch"]).astype(np.int64)
    descriptors = np.asarray(inputs["descriptors"], np.float32)

    in_maps, meta = _host_prep(
        inputs["x"], inputs["edge_index"], inputs["edge_attr"],
        inputs["node_W"], inputs["node_b"], inputs["edge_W"], inputs["edge_b"],
        inputs["emW1"], inputs["emb1"], inputs["emW2"], inputs["emb2"],
        inputs["umW1"], inputs["umb1"], inputs["umW2"], inputs["umb2"])

    nc = _build(meta)

    res = run_bass_kernel_spmd(nc, in_maps, core_ids=list(range(NCORES)))

    h = np.concatenate([res.results[c]["hT_out"].T for c in range(NCORES)],
                       axis=0)

    # host readout: mean-pool per graph + MLP + sigmoid (0.05% of FLOPs)
    sums = np.zeros((G, H), np.float32)
    np.add.at(sums, batch, h)
    cnt = np.bincount(batch, minlength=G).astype(np.float32)
    pooled = sums / np.maximum(cnt, 1.0)[:, None]
    r = np.concatenate([pooled, descriptors], axis=1)
    z = np.maximum(r @ ro_W1 + ro_b1, 0.0) @ ro_W2 + ro_b2
    out = 1.0 / (1.0 + np.exp(-z))
    return out.reshape(-1).astype(np.float32)
